# revision 13
# baseline (speedup 1.0000x reference)
"""Real spherical harmonics Y_l^m (l <= 10) for 1M points on 8 TRN2 NeuronCores.

Data-parallel: shard the 1M points across 8 cores (125000 each, laid out
[125 partitions x 1000 cols]).  Per core, compute all 121 channels with
fully-normalized associated-Legendre recurrences:

    G_lm(x) = Pbar_lm(x) / s^m   (polynomial in x; CS-phase sign folded out)
    col(l,+m) = G_lm * Cp_m,  Cp_m = sqrt2 * s^m * cos(m phi)
    col(l,-m) = G_lm * Sp_m,  Sp_m = sqrt2 * s^m * sin(m phi)
    col(l, 0) = G_l0

s^m computed via exact u = 1-x^2 products (one sqrt for odd m);
trig via ScalarE Sin with free affine (sin(m*phi + shift)).
"""

import math
import sys

import numpy as np

sys.path.insert(0, "/opt/trn_rl_repo")

import concourse.bass as bass  # noqa: E402
import concourse.mybir as mybir  # noqa: E402
from concourse.mybir import AluOpType  # noqa: E402
from concourse.tile import TileContext  # noqa: E402
from concourse.bass_utils import run_bass_kernel_spmd  # noqa: E402

from concourse.tile import TileContext as _TC  # noqa: E402


def _patched_drain_and_barrier(self, tick_clock, wait_clock):
    """Kernel-tail drain emitting at most ONE sem wait per TPB_CTRL
    instruction: this walrus build's CoreV2/V3 codegen rejects more
    ("Too many sync wait commands")."""
    from concourse.tile import ScopedClock

    nc = self.nc
    probe = nc.sync.nop(nofuse=True, hint="wait_split_probe")
    wait_clock.add_sem_waits(probe.ins, ScopedClock({None: tick_clock.global_clock}))
    si = probe.ins.sync_info
    waits = list(si.on_wait) if si is not None else []
    if len(waits) > 1:
        probe.ins.sync_info = mybir.SyncInfo(on_wait=waits[:1], on_update=[])
        for i, w in enumerate(waits[1:]):
            nop = nc.sync.nop(nofuse=True, hint=f"wait_split_{i}")
            nop.ins.sync_info = mybir.SyncInfo(on_wait=[w], on_update=[])
    nc.sync.drain()
    nc.all_engine_barrier()
    popped = nc._tile_sem_poison_stack.pop()
    assert popped is self._sem_poison
    nc.clear_and_free_semaphores(list(self.sems.allocated().values()))
    nc.all_engine_barrier()


_TC._drain_and_barrier = _patched_drain_and_barrier

# This walrus build's codegen accepts only ONE sem-wait per TPB engine
# instruction.  Split excess waits onto same-engine NoOps committed just
# before the instruction (engine streams execute in bb order per engine).
_WAIT_EXEMPT_OPCODES = set()
_orig_add_instruction = _TC._add_instruction


def _patched_add_instruction(self, inst):
    si = getattr(inst, "sync_info", None)
    if si is not None:
        waits = list(si.on_wait)
        if len(waits) > 1 and str(inst.opcode) not in _WAIT_EXEMPT_OPCODES:
            for j, w in enumerate(waits[1:]):
                nop = mybir.InstNoOp(
                    name=f"{inst.name}-wsplit{j}",
                    engine=inst.engine,
                    sync_info=mybir.SyncInfo(on_wait=[w], on_update=[]),
                    bass_nofuse=True,
                )
                _orig_add_instruction(self, nop)
            inst.sync_info = mybir.SyncInfo(
                on_wait=waits[:1], on_update=list(si.on_update)
            )
    _orig_add_instruction(self, inst)


_TC._add_instruction = _patched_add_instruction

L_MAX = 10
N_PTS = 1_000_000
N_CORES = 8
P = 125          # partitions used per core
W = 1000         # free-dim columns per core  (125*1000*8 = 1M)
NCH = (L_MAX + 1) ** 2  # 121 output channels

F32 = mybir.dt.float32

# chain dtype / output dtype knobs
CHAIN_DT = F32
OUT_DT = F32
USE_ACT_SIN = True   # cos/sin(m*phi) via ScalarE Sin LUT; False -> Chebyshev on DVE


def _coeffs():
    """Recurrence constants (float64 on host)."""
    gam = {0: math.sqrt(1.0 / (4.0 * math.pi))}
    for m in range(1, L_MAX + 1):
        gam[m] = math.sqrt((2 * m + 1) / (2.0 * m)) * gam[m - 1]

    def a_(l, m):
        return math.sqrt((4.0 * l * l - 1.0) / (l * l - m * m))

    def b_(l, m):
        return -math.sqrt(
            ((2.0 * l + 1.0) * ((l - 1.0) ** 2 - m * m))
            / ((2.0 * l - 3.0) * (l * l - m * m))
        )

    return gam, a_, b_


def _chan(l, m):
    return l * l + (m + l)


def build_nc():
    nc = bass.Bass(target_bir_lowering=False)
    x_d = nc.declare_dram_parameter("x", [P, W], F32, isOutput=False)
    phi_d = nc.declare_dram_parameter("phi", [P, W], F32, isOutput=False)
    out_d = nc.declare_dram_parameter("out", [NCH, P, W], OUT_DT, isOutput=True)

    gam, a_, b_ = _coeffs()
    sqrt2 = math.sqrt(2.0)

    with TileContext(nc) as tc:
        with tc.tile_pool(name="main", bufs=1) as pool, \
             tc.tile_pool(name="trig", bufs=2) as trig_pool, \
             tc.tile_pool(name="chain", bufs=4) as chain_pool, \
             tc.tile_pool(name="outp", bufs=10) as out_pool:

            xt = pool.tile([P, W], F32, name="xt")
            pt = pool.tile([P, W], F32, name="pt")
            nc.sync.dma_start(out=xt[:], in_=x_d[:, :])
            nc.sync.dma_start(out=pt[:], in_=phi_d[:, :])



            # u = 1 - x^2 (exact-ish fp32), s2 = sqrt(2u) = sqrt2 * s
            x2 = pool.tile([P, W], F32, name="x2")
            nc.vector.tensor_tensor(x2[:], xt[:], xt[:], AluOpType.mult)
            u = pool.tile([P, W], F32, name="u")
            nc.vector.tensor_scalar(u[:], x2[:], -1.0, 1.0,
                                    AluOpType.mult, AluOpType.add)
            s2 = pool.tile([P, W], F32, name="s2")
            nc.scalar.activation(s2[:], u[:], mybir.ActivationFunctionType.Sqrt,
                                 scale=2.0)

            # sin(phi) directly (|phi| <= pi: in the ACT Sin LUT range);
            # cos(phi) = 1 - 2*sin(phi/2)^2 (half-angle keeps the arg small).
            s1t = pool.tile([P, W], F32, name="s1t")
            nc.scalar.activation(s1t[:], pt[:], mybir.ActivationFunctionType.Sin)
            sh = pool.tile([P, W], F32, name="sh")
            nc.scalar.activation(sh[:], pt[:], mybir.ActivationFunctionType.Sin,
                                 scale=0.5)
            c1t = pool.tile([P, W], F32, name="c1t")
            nc.vector.tensor_tensor(c1t[:], sh[:], sh[:], AluOpType.mult)
            nc.vector.tensor_scalar(c1t[:], c1t[:], -2.0, 1.0,
                                    AluOpType.mult, AluOpType.add)
            t2 = pool.tile([P, W], F32, name="t2")
            nc.vector.tensor_scalar(t2[:], c1t[:], 2.0, None, AluOpType.mult)

            def out_tile(name):
                return out_pool.tile([P, W], OUT_DT, name=name, tag="out")

            def emit_out(l, m, t):
                nc.sync.dma_start(out=out_d[_chan(l, m), :, :], in_=t[:])

            # ---------------- m = 0 chain (outputs are the chain) ----------
            g_prev2 = out_tile("g00")
            nc.vector.memset(g_prev2[:], gam[0])
            emit_out(0, 0, g_prev2)
            g_prev = out_tile("g10")
            nc.scalar.mul(g_prev[:], xt[:], math.sqrt(3.0) * gam[0])
            emit_out(1, 0, g_prev)
            for l in range(2, L_MAX + 1):
                v = chain_pool.tile([P, W], CHAIN_DT, name=f"v0_{l}", tag="v")
                nc.vector.scalar_tensor_tensor(
                    v[:], g_prev[:], a_(l, 0), xt[:], AluOpType.mult, AluOpType.mult)
                g = out_tile(f"g{l}0")
                nc.vector.scalar_tensor_tensor(
                    g[:], g_prev2[:], b_(l, 0), v[:], AluOpType.mult, AluOpType.add)
                emit_out(l, 0, g)
                g_prev2, g_prev = g_prev, g

            # ---------------- m >= 1 chains -------------------------------
            d_prev2 = None  # D_{m-2} tile (sqrt2 * s^{m-2})
            d_prev = None
            c_prev2 = s_prev2 = None
            c_prev, s_prev = c1t, s1t
            for m in range(1, L_MAX + 1):
                # trig: cm = cos(m phi), sm = sin(m phi) via Chebyshev:
                #   f_m = 2*cos(phi)*f_{m-1} - f_{m-2}
                if m == 1:
                    cm, sm = c1t, s1t
                else:
                    cm = trig_pool.tile([P, W], F32, name=f"c{m}", tag="cm", bufs=3)
                    sm = trig_pool.tile([P, W], F32, name=f"s{m}", tag="sm", bufs=3)
                    if m == 2:
                        # c2 = 2c1^2-1 = t2*c1 - 1 ; s2 = t2*s1 - 0
                        nc.vector.scalar_tensor_tensor(
                            cm[:], c_prev[:], 1.0, t2[:],
                            AluOpType.mult, AluOpType.mult)
                        nc.vector.tensor_scalar(cm[:], cm[:], 1.0, None,
                                                AluOpType.subtract)
                        nc.vector.tensor_tensor(sm[:], t2[:], s_prev[:],
                                                AluOpType.mult)
                    else:
                        tmpc = trig_pool.tile([P, W], F32, name=f"tc{m}",
                                              tag="tmpc", bufs=2)
                        nc.vector.tensor_tensor(tmpc[:], t2[:], c_prev[:],
                                                AluOpType.mult)
                        nc.vector.tensor_tensor(cm[:], tmpc[:], c_prev2[:],
                                                AluOpType.subtract)
                        tmps = trig_pool.tile([P, W], F32, name=f"ts{m}",
                                              tag="tmps", bufs=2)
                        nc.vector.tensor_tensor(tmps[:], t2[:], s_prev[:],
                                                AluOpType.mult)
                        nc.vector.tensor_tensor(sm[:], tmps[:], s_prev2[:],
                                                AluOpType.subtract)
                c_prev2, c_prev = c_prev, cm
                s_prev2, s_prev = s_prev, sm

                # D_m = sqrt2 * s^m
                if m == 1:
                    dm = s2
                elif m == 2:
                    dm = trig_pool.tile([P, W], F32, name="d2", tag="dm", bufs=3)
                    nc.vector.tensor_scalar(dm[:], u[:], sqrt2, None, AluOpType.mult)
                else:
                    dm = trig_pool.tile([P, W], F32, name=f"d{m}", tag="dm", bufs=3)
                    nc.vector.tensor_tensor(dm[:], d_prev2[:], u[:], AluOpType.mult)
                d_prev2, d_prev = d_prev, dm

                cp = trig_pool.tile([P, W], CHAIN_DT, name=f"cp{m}", tag="cp")
                sp = trig_pool.tile([P, W], CHAIN_DT, name=f"sp{m}", tag="sp")
                nc.vector.tensor_tensor(cp[:], dm[:], cm[:], AluOpType.mult)
                nc.vector.tensor_tensor(sp[:], dm[:], sm[:], AluOpType.mult)

                # l = m diagonal: G_mm = gam[m] (constant)
                o1 = out_tile(f"o_{m}_{m}c")
                nc.scalar.mul(o1[:], cp[:], gam[m])
                emit_out(m, m, o1)
                o2 = out_tile(f"o_{m}_{m}s")
                nc.scalar.mul(o2[:], sp[:], gam[m])
                emit_out(m, -m, o2)

                if m == L_MAX:
                    break

                # l = m+1: G = sqrt(2m+3) * gam[m] * x
                g_prev2 = None
                g_prev = chain_pool.tile([P, W], CHAIN_DT, name=f"g_{m+1}_{m}",
                                         tag="g")
                nc.scalar.mul(g_prev[:], xt[:], math.sqrt(2.0 * m + 3.0) * gam[m])
                o1 = out_tile(f"o_{m+1}_{m}c")
                nc.vector.tensor_tensor(o1[:], g_prev[:], cp[:], AluOpType.mult)
                emit_out(m + 1, m, o1)
                o2 = out_tile(f"o_{m+1}_{m}s")
                nc.vector.tensor_tensor(o2[:], g_prev[:], sp[:], AluOpType.mult)
                emit_out(m + 1, -m, o2)
                g_prev2_val = gam[m]  # G_mm is a uniform constant

                for l in range(m + 2, L_MAX + 1):
                    v = chain_pool.tile([P, W], CHAIN_DT, name=f"v_{l}_{m}", tag="v")
                    nc.vector.scalar_tensor_tensor(
                        v[:], g_prev[:], a_(l, m), xt[:],
                        AluOpType.mult, AluOpType.mult)
                    g = chain_pool.tile([P, W], CHAIN_DT, name=f"g_{l}_{m}", tag="g")
                    if g_prev2 is None:
                        # G_{l-2} is the constant gam[m]: g = b*gam[m] + v
                        nc.vector.tensor_scalar(
                            g[:], v[:], b_(l, m) * g_prev2_val, None, AluOpType.add)
                    else:
                        nc.vector.scalar_tensor_tensor(
                            g[:], g_prev2[:], b_(l, m), v[:],
                            AluOpType.mult, AluOpType.add)
                    o1 = out_tile(f"o_{l}_{m}c")
                    nc.vector.tensor_tensor(o1[:], g[:], cp[:], AluOpType.mult)
                    emit_out(l, m, o1)
                    o2 = out_tile(f"o_{l}_{m}s")
                    nc.vector.tensor_tensor(o2[:], g[:], sp[:], AluOpType.mult)
                    emit_out(l, -m, o2)
                    g_prev2, g_prev = g_prev, g

    return nc


_NC_CACHE = None


def _get_nc():
    global _NC_CACHE
    if _NC_CACHE is None:
        _NC_CACHE = build_nc()
    return _NC_CACHE


def kernel(cos_theta: np.ndarray, phi: np.ndarray):
    nc = _get_nc()
    x = np.asarray(cos_theta, dtype=np.float32).reshape(N_CORES, P, W)
    p = np.asarray(phi, dtype=np.float32).reshape(N_CORES, P, W)
    in_maps = [{"x": x[i], "phi": p[i]} for i in range(N_CORES)]
    res = run_bass_kernel_spmd(nc, in_maps, core_ids=list(range(N_CORES)))
    outs = res.results
    # outs: list per core of dict name->array; "out" is [121, P, W]
    full = np.empty((N_PTS, NCH), dtype=np.float32)
    for i in range(N_CORES):
        o = np.asarray(outs[i]["out"], dtype=np.float32)  # [121, P, W]
        full[i * P * W:(i + 1) * P * W] = o.reshape(NCH, P * W).T
    result = []
    for l in range(L_MAX + 1):
        result.append(np.ascontiguousarray(full[:, l * l:(l + 1) * (l + 1)]))
    return tuple(result)


# revision 39
# speedup vs baseline: 30.5265x; 30.5265x over previous
"""Real spherical harmonics Y_l^m (l <= 10) for 1M points on 8 TRN2 NeuronCores.

Data-parallel: shard the 1M points across 8 cores (125000 each, laid out
[125 partitions x 1000 cols]).  Per core, compute all 121 channels with
fully-normalized associated-Legendre recurrences:

    G_lm(x) = Pbar_lm(x) / s^m   (polynomial in x; CS-phase sign folded out)
    col(l,+m) = G_lm * Cp_m,  Cp_m = sqrt2 * s^m * cos(m phi)
    col(l,-m) = G_lm * Sp_m,  Sp_m = sqrt2 * s^m * sin(m phi)
    col(l, 0) = G_l0

s^m computed via exact u = 1-x^2 products (one sqrt for odd m);
trig via ScalarE Sin with free affine (sin(m*phi + shift)).
"""

import math
import sys

import numpy as np

sys.path.insert(0, "/opt/trn_rl_repo")

import concourse.bass as bass  # noqa: E402
import concourse.mybir as mybir  # noqa: E402
from concourse.mybir import AluOpType  # noqa: E402
from concourse.tile import TileContext  # noqa: E402
from concourse.bass_utils import run_bass_kernel_spmd  # noqa: E402

from concourse.tile import TileContext as _TC  # noqa: E402


def _patched_drain_and_barrier(self, tick_clock, wait_clock):
    """Kernel-tail drain emitting at most ONE sem wait per TPB_CTRL
    instruction: this walrus build's CoreV2/V3 codegen rejects more
    ("Too many sync wait commands")."""
    from concourse.tile import ScopedClock

    nc = self.nc
    probe = nc.sync.nop(nofuse=True, hint="wait_split_probe")
    wait_clock.add_sem_waits(probe.ins, ScopedClock({None: tick_clock.global_clock}))
    si = probe.ins.sync_info
    waits = list(si.on_wait) if si is not None else []
    if len(waits) > 1:
        probe.ins.sync_info = mybir.SyncInfo(on_wait=waits[:1], on_update=[])
        for i, w in enumerate(waits[1:]):
            nop = nc.sync.nop(nofuse=True, hint=f"wait_split_{i}")
            nop.ins.sync_info = mybir.SyncInfo(on_wait=[w], on_update=[])
    nc.sync.drain()
    nc.all_engine_barrier()
    popped = nc._tile_sem_poison_stack.pop()
    assert popped is self._sem_poison
    nc.clear_and_free_semaphores(list(self.sems.allocated().values()))
    nc.all_engine_barrier()


_TC._drain_and_barrier = _patched_drain_and_barrier

# This walrus build's codegen accepts only ONE sem-wait per TPB engine
# instruction.  Split excess waits onto same-engine NoOps committed just
# before the instruction (engine streams execute in bb order per engine).
_WAIT_EXEMPT_OPCODES = set()
_orig_add_instruction = _TC._add_instruction


def _patched_add_instruction(self, inst):
    si = getattr(inst, "sync_info", None)
    if si is not None:
        waits = list(si.on_wait)
        if len(waits) > 1 and str(inst.opcode) not in _WAIT_EXEMPT_OPCODES:
            for j, w in enumerate(waits[1:]):
                nop = mybir.InstNoOp(
                    name=f"{inst.name}-wsplit{j}",
                    engine=inst.engine,
                    sync_info=mybir.SyncInfo(on_wait=[w], on_update=[]),
                    bass_nofuse=True,
                )
                _orig_add_instruction(self, nop)
            inst.sync_info = mybir.SyncInfo(
                on_wait=waits[:1], on_update=list(si.on_update)
            )
    _orig_add_instruction(self, inst)


_TC._add_instruction = _patched_add_instruction

L_MAX = 10
N_PTS = 1_000_000
N_CORES = 8
P = 125          # partitions used per core
W = 1000         # free-dim columns per core  (125*1000*8 = 1M)
NCH = (L_MAX + 1) ** 2  # 121 output channels

F32 = mybir.dt.float32

# chain dtype / output dtype knobs
CHAIN_DT = F32
OUT_DT = mybir.dt.float16
M16 = 3   # fp16 chains for m >= M16
C_CHUNK = 1
NCHUNKS = NCH // C_CHUNK
USE_ACT_SIN = True   # cos/sin(m*phi) via ScalarE Sin LUT; False -> Chebyshev on DVE


def _coeffs():
    """Recurrence constants (float64 on host)."""
    gam = {0: math.sqrt(1.0 / (4.0 * math.pi))}
    for m in range(1, L_MAX + 1):
        gam[m] = math.sqrt((2 * m + 1) / (2.0 * m)) * gam[m - 1]

    def a_(l, m):
        return math.sqrt((4.0 * l * l - 1.0) / (l * l - m * m))

    def b_(l, m):
        return -math.sqrt(
            ((2.0 * l + 1.0) * ((l - 1.0) ** 2 - m * m))
            / ((2.0 * l - 3.0) * (l * l - m * m))
        )

    return gam, a_, b_


def _chan(l, m):
    return l * l + (m + l)


# Bump on EVERY kernel change: the axon terminal caches executables by an
# HLO fingerprint that ignores the embedded BIR, so two kernel versions with
# identical I/O signatures collide.  The salt input's width makes signatures
# unique per (version, mode).
KERNEL_VERSION = 7
_MODE_ID = {"full": 0, "compute_only": 1, "dma_only": 2}


def _salt_width(mode, loop_iters, timing):
    return (KERNEL_VERSION * 37 + _MODE_ID[mode] * 7
            + (loop_iters or 0) % 29 + (3 if timing else 0)) % 480 + 16


def build_nc(loop_iters=None, trace_sim=False, mode="full", timing=False):
    nc = bass.Bass(target_bir_lowering=False)
    salt_w = _salt_width(mode, loop_iters, timing)
    nc._angular_salt_w = salt_w
    salt_d = nc.declare_dram_parameter("salt", [1, salt_w], F32, isOutput=False)
    x_d = nc.declare_dram_parameter("x", [P, W], F32, isOutput=False)
    phi_d = nc.declare_dram_parameter("phi", [P, W], F32, isOutput=False)
    if timing:
        # Internal scratch stands in for the real output: same HBM DMA
        # traffic, but no 484MB host zero-staging per run (timing noise).
        out_d = nc.dram_tensor("out_scratch", [NCHUNKS, P, C_CHUNK, W], OUT_DT)
        tiny_d = nc.declare_dram_parameter("out", [P, W], OUT_DT, isOutput=True)
    else:
        out_d = nc.declare_dram_parameter("out", [NCHUNKS, P, C_CHUNK, W],
                                          OUT_DT, isOutput=True)
        tiny_d = None

    gam, a_, b_ = _coeffs()
    sqrt2 = math.sqrt(2.0)

    with TileContext(nc, trace_sim=trace_sim) as tc:
        with tc.tile_pool(name="salt", bufs=1) as salt_pool:
            st = salt_pool.tile([1, salt_w], F32, name="salt_t")
            nc.sync.dma_start(out=st[:], in_=salt_d[:, :])
        with tc.tile_pool(name="main", bufs=1) as pool, \
             tc.tile_pool(name="trig", bufs=2) as trig_pool, \
             tc.tile_pool(name="chain", bufs=4) as chain_pool, \
             tc.tile_pool(name="outp", bufs=10 if C_CHUNK == 1 else 3) as out_pool:
            if loop_iters is not None:
                import contextlib
                loop_cm = tc.For_i(0, loop_iters, 1)
            else:
                import contextlib
                loop_cm = contextlib.nullcontext()
            with loop_cm:
                if mode == "dma_only":
                    _emit_dma_only(nc, pool, out_pool, x_d, out_d)
                else:
                    _emit_body(nc, tc, pool, trig_pool, chain_pool, out_pool,
                               x_d, phi_d, out_d, gam, a_, b_, sqrt2,
                               skip_out_dma=(mode == "compute_only"))
            if tiny_d is not None:
                tt_ = pool.tile([P, W], OUT_DT, name="tiny_copy")
                nc.sync.dma_start(out=tt_[:], in_=out_d[0, :, 0, :])
                nc.sync.dma_start(out=tiny_d[:, :], in_=tt_[:])
    return nc


def _emit_dma_only(nc, pool, out_pool, x_d, out_d):
    src = out_pool.tile([P, C_CHUNK * W], OUT_DT, name="src")
    nc.vector.memset(src[:], 0.25)
    big = src[:].rearrange("p (c w) -> p c w", c=C_CHUNK)
    for j in range(NCHUNKS):
        nc.sync.dma_start(out=out_d[j, :, :, :], in_=big)


def _emit_body(nc, tc, pool, trig_pool, chain_pool, out_pool,
               x_d, phi_d, out_d, gam, a_, b_, sqrt2, skip_out_dma=False):
    F16 = mybir.dt.float16

    xt = pool.tile([P, W], F32, name="xt")
    pt = pool.tile([P, W], F32, name="pt")
    nc.sync.dma_start(out=xt[:], in_=x_d[:, :])
    nc.sync.dma_start(out=pt[:], in_=phi_d[:, :])

    # u = 1 - x^2 (fp32), s2 = sqrt(2u) = sqrt2 * s
    x2 = trig_pool.tile([P, W], F32, name="x2", tag="tmpc", bufs=2)
    nc.vector.tensor_tensor(x2[:], xt[:], xt[:], AluOpType.mult)
    u = pool.tile([P, W], F32, name="u")
    nc.vector.tensor_scalar(u[:], x2[:], -1.0, 1.0, AluOpType.mult, AluOpType.add)
    s2 = pool.tile([P, W], F32, name="s2")
    nc.scalar.activation(s2[:], u[:], mybir.ActivationFunctionType.Sqrt, scale=2.0)

    # sin(phi) in-range; cos(phi) = 1 - 2*sin(phi/2)^2 (half-angle)
    s1t = pool.tile([P, W], F32, name="s1t")
    nc.scalar.activation(s1t[:], pt[:], mybir.ActivationFunctionType.Sin)
    sh = trig_pool.tile([P, W], F32, name="sh", tag="tmps", bufs=2)
    nc.scalar.activation(sh[:], pt[:], mybir.ActivationFunctionType.Sin, scale=0.5)
    c1t = pool.tile([P, W], F32, name="c1t")
    nc.vector.tensor_tensor(c1t[:], sh[:], sh[:], AluOpType.mult)
    nc.vector.tensor_scalar(c1t[:], c1t[:], -2.0, 1.0, AluOpType.mult, AluOpType.add)
    t2 = pool.tile([P, W], F32, name="t2")
    nc.vector.tensor_scalar(t2[:], c1t[:], 2.0, None, AluOpType.mult)
    # fp16 copy of x for the fp16 chains
    xh = pool.tile([P, W], F16, name="xh")
    nc.vector.tensor_copy(xh[:], xt[:])

    # Output slices in PRODUCTION order, batched C_CHUNK channels per DMA.
    state = {"i": 0, "tile": None}

    def out_slice():
        j = state["i"]
        chunk, sl = divmod(j, C_CHUNK)
        if sl == 0:
            state["tile"] = out_pool.tile([P, C_CHUNK * W], OUT_DT,
                                          name=f"obig{chunk}", tag="out")
        state["i"] += 1
        return state["tile"][:, sl * W:(sl + 1) * W]

    def close_slice():
        j = state["i"]
        chunk, sl = divmod(j, C_CHUNK)
        if sl == 0 and not skip_out_dma:
            big = state["tile"][:].rearrange("p (c w) -> p c w", c=C_CHUNK)
            nc.sync.dma_start(out=out_d[chunk - 1, :, :, :], in_=big)

    # ---------------- m = 0 chain (fp32, ACT copies to fp16 slices) -----
    o = out_slice()
    nc.vector.memset(o, gam[0])
    close_slice()
    g_prev2 = None
    g_prev2_val = gam[0]
    g_prev = chain_pool.tile([P, W], F32, name="g10", tag="g", bufs=3)
    nc.scalar.mul(g_prev[:], xt[:], math.sqrt(3.0) * gam[0])
    o = out_slice()
    nc.vector.tensor_copy(o, g_prev[:])
    close_slice()
    for l in range(2, L_MAX + 1):
        v = chain_pool.tile([P, W], F32, name=f"v0_{l}", tag="v", bufs=2)
        nc.vector.scalar_tensor_tensor(
            v[:], g_prev[:], a_(l, 0), xt[:], AluOpType.mult, AluOpType.mult)
        g = chain_pool.tile([P, W], F32, name=f"g{l}0", tag="g", bufs=3)
        if g_prev2 is None:
            nc.vector.tensor_scalar(
                g[:], v[:], b_(l, 0) * g_prev2_val, None, AluOpType.add)
        else:
            nc.vector.scalar_tensor_tensor(
                g[:], g_prev2[:], b_(l, 0), v[:], AluOpType.mult, AluOpType.add)
        o = out_slice()
        nc.vector.tensor_copy(o, g[:])
        close_slice()
        g_prev2, g_prev = g_prev, g

    # ---------------- m >= 1 chains -------------------------------------
    d_prev2 = None
    d_prev = None
    c_prev2 = s_prev2 = None
    c_prev, s_prev = c1t, s1t
    for m in range(1, L_MAX + 1):
        fp16_chain = m >= M16
        cdt = F16 if fp16_chain else F32
        xop = xh if fp16_chain else xt
        gtag = "gh" if fp16_chain else "g"
        vtag = "vh" if fp16_chain else "v"

        # trig via Chebyshev (fp32): f_m = 2cos(phi) f_{m-1} - f_{m-2}
        if m == 1:
            cm, sm = c1t, s1t
        else:
            cm = trig_pool.tile([P, W], F32, name=f"c{m}", tag="cm", bufs=3)
            sm = trig_pool.tile([P, W], F32, name=f"s{m}", tag="sm", bufs=3)
            if m == 2:
                nc.vector.scalar_tensor_tensor(
                    cm[:], c_prev[:], 1.0, t2[:], AluOpType.mult, AluOpType.mult)
                nc.vector.tensor_scalar(cm[:], cm[:], 1.0, None,
                                        AluOpType.subtract)
                nc.vector.tensor_tensor(sm[:], t2[:], s_prev[:], AluOpType.mult)
            else:
                tmpc = trig_pool.tile([P, W], F32, name=f"tc{m}", tag="tmpc",
                                      bufs=2)
                nc.vector.tensor_tensor(tmpc[:], t2[:], c_prev[:], AluOpType.mult)
                nc.vector.tensor_tensor(cm[:], tmpc[:], c_prev2[:],
                                        AluOpType.subtract)
                tmps = trig_pool.tile([P, W], F32, name=f"ts{m}", tag="tmps",
                                      bufs=2)
                nc.vector.tensor_tensor(tmps[:], t2[:], s_prev[:], AluOpType.mult)
                nc.vector.tensor_tensor(sm[:], tmps[:], s_prev2[:],
                                        AluOpType.subtract)
        c_prev2, c_prev = c_prev, cm
        s_prev2, s_prev = s_prev, sm

        # D_m = sqrt2 * s^m (even powers from u exactly; one sqrt for odd)
        if m == 1:
            dm = s2
        elif m == 2:
            dm = trig_pool.tile([P, W], F32, name="d2", tag="dm", bufs=3)
            nc.vector.tensor_scalar(dm[:], u[:], sqrt2, None, AluOpType.mult)
        else:
            dm = trig_pool.tile([P, W], F32, name=f"d{m}", tag="dm", bufs=3)
            nc.vector.tensor_tensor(dm[:], d_prev2[:], u[:], AluOpType.mult)
        d_prev2, d_prev = d_prev, dm

        cp = trig_pool.tile([P, W], cdt, name=f"cp{m}",
                            tag="cph" if fp16_chain else "cp", bufs=2)
        sp = trig_pool.tile([P, W], cdt, name=f"sp{m}",
                            tag="sph" if fp16_chain else "sp", bufs=2)
        nc.vector.tensor_tensor(cp[:], dm[:], cm[:], AluOpType.mult)
        nc.vector.tensor_tensor(sp[:], dm[:], sm[:], AluOpType.mult)

        # l = m diagonal (G_mm = gam[m] constant).  ACT only for same-dtype
        # writes (its Copy does NOT cast); DVE casts on write.
        o = out_slice()
        if fp16_chain:
            nc.scalar.mul(o, cp[:], gam[m])
        else:
            nc.vector.tensor_scalar(o, cp[:], gam[m], None, AluOpType.mult)
        close_slice()
        o = out_slice()
        if fp16_chain:
            nc.scalar.mul(o, sp[:], gam[m])
        else:
            nc.vector.tensor_scalar(o, sp[:], gam[m], None, AluOpType.mult)
        close_slice()

        if m == L_MAX:
            break

        # l = m+1: G = sqrt(2m+3) gam[m] x
        g_prev2 = None
        g_prev2_val = gam[m]
        g_prev = chain_pool.tile([P, W], cdt, name=f"g_{m+1}_{m}", tag=gtag,
                                 bufs=3)
        if fp16_chain:
            nc.vector.tensor_scalar(g_prev[:], xh[:],
                                    math.sqrt(2.0 * m + 3.0) * gam[m], None,
                                    AluOpType.mult)
        else:
            nc.scalar.mul(g_prev[:], xt[:], math.sqrt(2.0 * m + 3.0) * gam[m])
        o = out_slice()
        nc.vector.tensor_tensor(o, g_prev[:], cp[:], AluOpType.mult)
        close_slice()
        o = out_slice()
        nc.vector.tensor_tensor(o, g_prev[:], sp[:], AluOpType.mult)
        close_slice()

        for l in range(m + 2, L_MAX + 1):
            v = chain_pool.tile([P, W], cdt, name=f"v_{l}_{m}", tag=vtag, bufs=2)
            nc.vector.scalar_tensor_tensor(
                v[:], g_prev[:], a_(l, m), xop[:], AluOpType.mult, AluOpType.mult)
            g = chain_pool.tile([P, W], cdt, name=f"g_{l}_{m}", tag=gtag, bufs=3)
            if g_prev2 is None:
                nc.vector.tensor_scalar(
                    g[:], v[:], b_(l, m) * g_prev2_val, None, AluOpType.add)
            else:
                nc.vector.scalar_tensor_tensor(
                    g[:], g_prev2[:], b_(l, m), v[:], AluOpType.mult,
                    AluOpType.add)
            o = out_slice()
            nc.vector.tensor_tensor(o, g[:], cp[:], AluOpType.mult)
            close_slice()
            o = out_slice()
            nc.vector.tensor_tensor(o, g[:], sp[:], AluOpType.mult)
            close_slice()
            g_prev2, g_prev = g_prev, g

    # flush the final chunk
    j = state["i"]
    assert j == NCH, j
    if not skip_out_dma:
        big = state["tile"][:].rearrange("p (c w) -> p c w", c=C_CHUNK)
        nc.sync.dma_start(out=out_d[NCHUNKS - 1, :, :, :], in_=big)


_NC_CACHE = None


def _get_nc():
    global _NC_CACHE
    if _NC_CACHE is None:
        _NC_CACHE = build_nc()
    return _NC_CACHE


def _production_order():
    """(l, m) per output channel in the order the device emits them."""
    lst = [(l, 0) for l in range(L_MAX + 1)]
    for m in range(1, L_MAX + 1):
        lst.append((m, m))
        lst.append((m, -m))
        if m < L_MAX:
            lst.append((m + 1, m))
            lst.append((m + 1, -m))
        for l in range(m + 2, L_MAX + 1):
            lst.append((l, m))
            lst.append((l, -m))
    assert len(lst) == NCH
    return lst


def kernel(cos_theta: np.ndarray, phi: np.ndarray):
    nc = _get_nc()
    x = np.asarray(cos_theta, dtype=np.float32).reshape(N_CORES, P, W)
    p = np.asarray(phi, dtype=np.float32).reshape(N_CORES, P, W)
    salt = np.zeros((1, nc._angular_salt_w), np.float32)
    in_maps = [{"x": x[i], "phi": p[i], "salt": salt} for i in range(N_CORES)]
    res = run_bass_kernel_spmd(nc, in_maps, core_ids=list(range(N_CORES)))
    outs = res.results
    # outs: per core "out" is [121, P, W] in production order
    order = _production_order()
    chan_of_seq = np.array([_chan(l, m) for (l, m) in order])
    perm = np.empty(NCH, dtype=np.int64)
    perm[chan_of_seq] = np.arange(NCH)  # perm[chan] = seq index
    full = np.empty((N_PTS, NCH), dtype=np.float32)
    for i in range(N_CORES):
        o = np.asarray(outs[i]["out"]).astype(np.float32)  # [11, P, 11, W]
        o = o.transpose(0, 2, 1, 3).reshape(NCH, P * W)     # seq-major
        full[i * P * W:(i + 1) * P * W] = o.T[:, perm]
    result = []
    for l in range(L_MAX + 1):
        result.append(np.ascontiguousarray(full[:, l * l:(l + 1) * (l + 1)]))
    return tuple(result)


# revision 41
# speedup vs baseline: 32.6572x; 1.0698x over previous
"""Real spherical harmonics Y_l^m (l <= 10) for 1M points on 8 TRN2 NeuronCores.

Data-parallel: shard the 1M points across 8 cores (125000 each, laid out
[125 partitions x 1000 cols]).  Per core, compute all 121 channels with
fully-normalized associated-Legendre recurrences:

    G_lm(x) = Pbar_lm(x) / s^m   (polynomial in x; CS-phase sign folded out)
    col(l,+m) = G_lm * Cp_m,  Cp_m = sqrt2 * s^m * cos(m phi)
    col(l,-m) = G_lm * Sp_m,  Sp_m = sqrt2 * s^m * sin(m phi)
    col(l, 0) = G_l0

s^m computed via exact u = 1-x^2 products (one sqrt for odd m);
trig via ScalarE Sin with free affine (sin(m*phi + shift)).
"""

import math
import sys

import numpy as np

sys.path.insert(0, "/opt/trn_rl_repo")

import concourse.bass as bass  # noqa: E402
import concourse.mybir as mybir  # noqa: E402
from concourse.mybir import AluOpType  # noqa: E402
from concourse.tile import TileContext  # noqa: E402
from concourse.bass_utils import run_bass_kernel_spmd  # noqa: E402

from concourse.tile import TileContext as _TC  # noqa: E402


def _patched_drain_and_barrier(self, tick_clock, wait_clock):
    """Kernel-tail drain emitting at most ONE sem wait per TPB_CTRL
    instruction: this walrus build's CoreV2/V3 codegen rejects more
    ("Too many sync wait commands")."""
    from concourse.tile import ScopedClock

    nc = self.nc
    probe = nc.sync.nop(nofuse=True, hint="wait_split_probe")
    wait_clock.add_sem_waits(probe.ins, ScopedClock({None: tick_clock.global_clock}))
    si = probe.ins.sync_info
    waits = list(si.on_wait) if si is not None else []
    if len(waits) > 1:
        probe.ins.sync_info = mybir.SyncInfo(on_wait=waits[:1], on_update=[])
        for i, w in enumerate(waits[1:]):
            nop = nc.sync.nop(nofuse=True, hint=f"wait_split_{i}")
            nop.ins.sync_info = mybir.SyncInfo(on_wait=[w], on_update=[])
    nc.sync.drain()
    nc.all_engine_barrier()
    popped = nc._tile_sem_poison_stack.pop()
    assert popped is self._sem_poison
    nc.clear_and_free_semaphores(list(self.sems.allocated().values()))
    nc.all_engine_barrier()


_TC._drain_and_barrier = _patched_drain_and_barrier

# This walrus build's codegen accepts only ONE sem-wait per TPB engine
# instruction.  Split excess waits onto same-engine NoOps committed just
# before the instruction (engine streams execute in bb order per engine).
_WAIT_EXEMPT_OPCODES = set()
_orig_add_instruction = _TC._add_instruction


def _patched_add_instruction(self, inst):
    si = getattr(inst, "sync_info", None)
    if si is not None:
        waits = list(si.on_wait)
        if len(waits) > 1 and str(inst.opcode) not in _WAIT_EXEMPT_OPCODES:
            for j, w in enumerate(waits[1:]):
                nop = mybir.InstNoOp(
                    name=f"{inst.name}-wsplit{j}",
                    engine=inst.engine,
                    sync_info=mybir.SyncInfo(on_wait=[w], on_update=[]),
                    bass_nofuse=True,
                )
                _orig_add_instruction(self, nop)
            inst.sync_info = mybir.SyncInfo(
                on_wait=waits[:1], on_update=list(si.on_update)
            )
    _orig_add_instruction(self, inst)


_TC._add_instruction = _patched_add_instruction

L_MAX = 10
N_PTS = 1_000_000
N_CORES = 8
P = 125          # partitions used per core
W = 1000         # free-dim columns per core  (125*1000*8 = 1M)
NCH = (L_MAX + 1) ** 2  # 121 output channels

F32 = mybir.dt.float32

# chain dtype / output dtype knobs
CHAIN_DT = F32
OUT_DT = mybir.dt.float16
M16 = 3   # fp16 chains for m >= M16
C_CHUNK = 1
NCHUNKS = NCH // C_CHUNK
USE_ACT_SIN = True   # cos/sin(m*phi) via ScalarE Sin LUT; False -> Chebyshev on DVE


def _coeffs():
    """Recurrence constants (float64 on host)."""
    gam = {0: math.sqrt(1.0 / (4.0 * math.pi))}
    for m in range(1, L_MAX + 1):
        gam[m] = math.sqrt((2 * m + 1) / (2.0 * m)) * gam[m - 1]

    def a_(l, m):
        return math.sqrt((4.0 * l * l - 1.0) / (l * l - m * m))

    def b_(l, m):
        return -math.sqrt(
            ((2.0 * l + 1.0) * ((l - 1.0) ** 2 - m * m))
            / ((2.0 * l - 3.0) * (l * l - m * m))
        )

    return gam, a_, b_


def _chan(l, m):
    return l * l + (m + l)


# Bump on EVERY kernel change: the axon terminal caches executables by an
# HLO fingerprint that ignores the embedded BIR, so two kernel versions with
# identical I/O signatures collide.  The salt input's width makes signatures
# unique per (version, mode).
KERNEL_VERSION = 7
_MODE_ID = {"full": 0, "compute_only": 1, "dma_only": 2}


def _salt_width(mode, loop_iters, timing):
    return (KERNEL_VERSION * 37 + _MODE_ID[mode] * 7
            + (loop_iters or 0) % 29 + (3 if timing else 0)) % 480 + 16


def build_nc(loop_iters=None, trace_sim=False, mode="full", timing=False):
    nc = bass.Bass(target_bir_lowering=False)
    salt_w = _salt_width(mode, loop_iters, timing)
    nc._angular_salt_w = salt_w
    salt_d = nc.declare_dram_parameter("salt", [1, salt_w], F32, isOutput=False)
    x_d = nc.declare_dram_parameter("x", [P, W], F32, isOutput=False)
    phi_d = nc.declare_dram_parameter("phi", [P, W], F32, isOutput=False)
    if timing:
        # Internal scratch stands in for the real output: same HBM DMA
        # traffic, but no 484MB host zero-staging per run (timing noise).
        out_d = nc.dram_tensor("out_scratch", [NCHUNKS, P, C_CHUNK, W], OUT_DT)
        tiny_d = nc.declare_dram_parameter("out", [P, W], OUT_DT, isOutput=True)
    else:
        out_d = nc.declare_dram_parameter("out", [NCHUNKS, P, C_CHUNK, W],
                                          OUT_DT, isOutput=True)
        tiny_d = None

    gam, a_, b_ = _coeffs()
    sqrt2 = math.sqrt(2.0)

    with TileContext(nc, trace_sim=trace_sim) as tc:
        with tc.tile_pool(name="salt", bufs=1) as salt_pool:
            st = salt_pool.tile([1, salt_w], F32, name="salt_t")
            nc.sync.dma_start(out=st[:], in_=salt_d[:, :])
        with tc.tile_pool(name="main", bufs=1) as pool, \
             tc.tile_pool(name="trig", bufs=2) as trig_pool, \
             tc.tile_pool(name="chain", bufs=4) as chain_pool, \
             tc.tile_pool(name="outp", bufs=10 if C_CHUNK == 1 else 3) as out_pool:
            if loop_iters is not None:
                import contextlib
                loop_cm = tc.For_i(0, loop_iters, 1)
            else:
                import contextlib
                loop_cm = contextlib.nullcontext()
            with loop_cm:
                if mode == "dma_only":
                    _emit_dma_only(nc, pool, out_pool, x_d, out_d)
                else:
                    _emit_body(nc, tc, pool, trig_pool, chain_pool, out_pool,
                               x_d, phi_d, out_d, gam, a_, b_, sqrt2,
                               skip_out_dma=(mode == "compute_only"))
            if tiny_d is not None:
                tt_ = pool.tile([P, W], OUT_DT, name="tiny_copy")
                nc.sync.dma_start(out=tt_[:], in_=out_d[0, :, 0, :])
                nc.sync.dma_start(out=tiny_d[:, :], in_=tt_[:])
    return nc


def _emit_dma_only(nc, pool, out_pool, x_d, out_d):
    src = out_pool.tile([P, C_CHUNK * W], OUT_DT, name="src")
    nc.vector.memset(src[:], 0.25)
    big = src[:].rearrange("p (c w) -> p c w", c=C_CHUNK)
    for j in range(NCHUNKS):
        nc.sync.dma_start(out=out_d[j, :, :, :], in_=big)


def _emit_body(nc, tc, pool, trig_pool, chain_pool, out_pool,
               x_d, phi_d, out_d, gam, a_, b_, sqrt2, skip_out_dma=False):
    F16 = mybir.dt.float16

    xt = pool.tile([P, W], F32, name="xt")
    pt = pool.tile([P, W], F32, name="pt")
    nc.sync.dma_start(out=xt[:], in_=x_d[:, :])
    nc.sync.dma_start(out=pt[:], in_=phi_d[:, :])

    # u = 1 - x^2 (fp32), s2 = sqrt(2u) = sqrt2 * s
    x2 = trig_pool.tile([P, W], F32, name="x2", tag="tmpc", bufs=2)
    nc.vector.tensor_tensor(x2[:], xt[:], xt[:], AluOpType.mult)
    u = pool.tile([P, W], F32, name="u")
    nc.vector.tensor_scalar(u[:], x2[:], -1.0, 1.0, AluOpType.mult, AluOpType.add)
    s2 = pool.tile([P, W], F32, name="s2")
    nc.scalar.activation(s2[:], u[:], mybir.ActivationFunctionType.Sqrt, scale=2.0)

    # sin(phi) in-range; cos(phi) = 1 - 2*sin(phi/2)^2 (half-angle)
    s1t = pool.tile([P, W], F32, name="s1t")
    nc.scalar.activation(s1t[:], pt[:], mybir.ActivationFunctionType.Sin)
    sh = trig_pool.tile([P, W], F32, name="sh", tag="tmps", bufs=2)
    nc.scalar.activation(sh[:], pt[:], mybir.ActivationFunctionType.Sin, scale=0.5)
    c1t = pool.tile([P, W], F32, name="c1t")
    nc.vector.tensor_tensor(c1t[:], sh[:], sh[:], AluOpType.mult)
    nc.vector.tensor_scalar(c1t[:], c1t[:], -2.0, 1.0, AluOpType.mult, AluOpType.add)
    t2 = pool.tile([P, W], F32, name="t2")
    nc.vector.tensor_scalar(t2[:], c1t[:], 2.0, None, AluOpType.mult)
    # fp16 copy of x for the fp16 chains
    xh = pool.tile([P, W], F16, name="xh")
    nc.vector.tensor_copy(xh[:], xt[:])

    # Output slices in PRODUCTION order, batched C_CHUNK channels per DMA.
    state = {"i": 0, "tile": None}

    def out_slice():
        j = state["i"]
        chunk, sl = divmod(j, C_CHUNK)
        if sl == 0:
            state["tile"] = out_pool.tile([P, C_CHUNK * W], OUT_DT,
                                          name=f"obig{chunk}", tag="out")
        state["i"] += 1
        return state["tile"][:, sl * W:(sl + 1) * W]

    def close_slice():
        j = state["i"]
        chunk, sl = divmod(j, C_CHUNK)
        if sl == 0 and not skip_out_dma:
            big = state["tile"][:].rearrange("p (c w) -> p c w", c=C_CHUNK)
            nc.sync.dma_start(out=out_d[chunk - 1, :, :, :], in_=big)

    # ---------------- m = 0 chain (fp32, ACT copies to fp16 slices) -----
    o = out_slice()
    nc.vector.memset(o, gam[0])
    close_slice()
    g_prev2 = None
    g_prev2_val = gam[0]
    g_prev = chain_pool.tile([P, W], F32, name="g10", tag="g", bufs=3)
    nc.scalar.mul(g_prev[:], xt[:], math.sqrt(3.0) * gam[0])
    o = out_slice()
    nc.vector.tensor_copy(o, g_prev[:])
    close_slice()
    for l in range(2, L_MAX + 1):
        v = chain_pool.tile([P, W], F32, name=f"v0_{l}", tag="v", bufs=2)
        nc.vector.scalar_tensor_tensor(
            v[:], g_prev[:], a_(l, 0), xt[:], AluOpType.mult, AluOpType.mult)
        g = chain_pool.tile([P, W], F32, name=f"g{l}0", tag="g", bufs=3)
        if g_prev2 is None:
            nc.vector.tensor_scalar(
                g[:], v[:], b_(l, 0) * g_prev2_val, None, AluOpType.add)
        else:
            nc.vector.scalar_tensor_tensor(
                g[:], g_prev2[:], b_(l, 0), v[:], AluOpType.mult, AluOpType.add)
        o = out_slice()
        nc.vector.tensor_copy(o, g[:])
        close_slice()
        g_prev2, g_prev = g_prev, g

    # ---------------- m >= 1 chains -------------------------------------
    d_prev2 = None
    d_prev = None
    c_prev2 = s_prev2 = None
    c_prev, s_prev = c1t, s1t
    for m in range(1, L_MAX + 1):
        fp16_chain = m >= M16
        cdt = F16 if fp16_chain else F32
        xop = xh if fp16_chain else xt
        gtag = "gh" if fp16_chain else "g"
        vtag = "vh" if fp16_chain else "v"

        # trig via Chebyshev (fp32): f_m = 2cos(phi) f_{m-1} - f_{m-2}
        if m == 1:
            cm, sm = c1t, s1t
        else:
            cm = trig_pool.tile([P, W], F32, name=f"c{m}", tag="cm", bufs=3)
            sm = trig_pool.tile([P, W], F32, name=f"s{m}", tag="sm", bufs=3)
            if m == 2:
                nc.vector.scalar_tensor_tensor(
                    cm[:], c_prev[:], 1.0, t2[:], AluOpType.mult, AluOpType.mult)
                nc.vector.tensor_scalar(cm[:], cm[:], 1.0, None,
                                        AluOpType.subtract)
                nc.vector.tensor_tensor(sm[:], t2[:], s_prev[:], AluOpType.mult)
            else:
                tmpc = trig_pool.tile([P, W], F32, name=f"tc{m}", tag="tmpc",
                                      bufs=2)
                nc.vector.tensor_tensor(tmpc[:], t2[:], c_prev[:], AluOpType.mult)
                nc.vector.tensor_tensor(cm[:], tmpc[:], c_prev2[:],
                                        AluOpType.subtract)
                tmps = trig_pool.tile([P, W], F32, name=f"ts{m}", tag="tmps",
                                      bufs=2)
                nc.vector.tensor_tensor(tmps[:], t2[:], s_prev[:], AluOpType.mult)
                nc.vector.tensor_tensor(sm[:], tmps[:], s_prev2[:],
                                        AluOpType.subtract)
        c_prev2, c_prev = c_prev, cm
        s_prev2, s_prev = s_prev, sm

        # D_m = sqrt2 * s^m (even powers from u exactly; one sqrt for odd)
        if m == 1:
            dm = s2
        elif m == 2:
            dm = trig_pool.tile([P, W], F32, name="d2", tag="dm", bufs=3)
            nc.vector.tensor_scalar(dm[:], u[:], sqrt2, None, AluOpType.mult)
        else:
            dm = trig_pool.tile([P, W], F32, name=f"d{m}", tag="dm", bufs=3)
            nc.vector.tensor_tensor(dm[:], d_prev2[:], u[:], AluOpType.mult)
        d_prev2, d_prev = d_prev, dm

        cp = trig_pool.tile([P, W], cdt, name=f"cp{m}",
                            tag="cph" if fp16_chain else "cp", bufs=2)
        sp = trig_pool.tile([P, W], cdt, name=f"sp{m}",
                            tag="sph" if fp16_chain else "sp", bufs=2)
        nc.vector.tensor_tensor(cp[:], dm[:], cm[:], AluOpType.mult)
        nc.vector.tensor_tensor(sp[:], dm[:], sm[:], AluOpType.mult)

        # l = m diagonal (G_mm = gam[m] constant).  ACT only for same-dtype
        # writes (its Copy does NOT cast); DVE casts on write.
        o = out_slice()
        if fp16_chain:
            nc.scalar.mul(o, cp[:], gam[m])
        else:
            nc.vector.tensor_scalar(o, cp[:], gam[m], None, AluOpType.mult)
        close_slice()
        o = out_slice()
        if fp16_chain:
            nc.scalar.mul(o, sp[:], gam[m])
        else:
            nc.vector.tensor_scalar(o, sp[:], gam[m], None, AluOpType.mult)
        close_slice()

        if m == L_MAX:
            break

        # l = m+1: G = sqrt(2m+3) gam[m] x
        g_prev2 = None
        g_prev2_val = gam[m]
        g_prev = chain_pool.tile([P, W], cdt, name=f"g_{m+1}_{m}", tag=gtag,
                                 bufs=3)
        if fp16_chain:
            nc.vector.tensor_scalar(g_prev[:], xh[:],
                                    math.sqrt(2.0 * m + 3.0) * gam[m], None,
                                    AluOpType.mult)
        else:
            nc.scalar.mul(g_prev[:], xt[:], math.sqrt(2.0 * m + 3.0) * gam[m])
        o = out_slice()
        nc.vector.tensor_tensor(o, g_prev[:], cp[:], AluOpType.mult)
        close_slice()
        o = out_slice()
        nc.vector.tensor_tensor(o, g_prev[:], sp[:], AluOpType.mult)
        close_slice()

        for l in range(m + 2, L_MAX + 1):
            v = chain_pool.tile([P, W], cdt, name=f"v_{l}_{m}", tag=vtag, bufs=2)
            nc.vector.scalar_tensor_tensor(
                v[:], g_prev[:], a_(l, m), xop[:], AluOpType.mult, AluOpType.mult)
            g = chain_pool.tile([P, W], cdt, name=f"g_{l}_{m}", tag=gtag, bufs=3)
            if g_prev2 is None:
                nc.vector.tensor_scalar(
                    g[:], v[:], b_(l, m) * g_prev2_val, None, AluOpType.add)
            else:
                nc.vector.scalar_tensor_tensor(
                    g[:], g_prev2[:], b_(l, m), v[:], AluOpType.mult,
                    AluOpType.add)
            o = out_slice()
            nc.vector.tensor_tensor(o, g[:], cp[:], AluOpType.mult)
            close_slice()
            o = out_slice()
            nc.vector.tensor_tensor(o, g[:], sp[:], AluOpType.mult)
            close_slice()
            g_prev2, g_prev = g_prev, g

    # flush the final chunk
    j = state["i"]
    assert j == NCH, j
    if not skip_out_dma:
        big = state["tile"][:].rearrange("p (c w) -> p c w", c=C_CHUNK)
        nc.sync.dma_start(out=out_d[NCHUNKS - 1, :, :, :], in_=big)


_NC_CACHE = None


def _get_nc():
    global _NC_CACHE
    if _NC_CACHE is None:
        _NC_CACHE = build_nc()
    return _NC_CACHE


def _production_order():
    """(l, m) per output channel in the order the device emits them."""
    lst = [(l, 0) for l in range(L_MAX + 1)]
    for m in range(1, L_MAX + 1):
        lst.append((m, m))
        lst.append((m, -m))
        if m < L_MAX:
            lst.append((m + 1, m))
            lst.append((m + 1, -m))
        for l in range(m + 2, L_MAX + 1):
            lst.append((l, m))
            lst.append((l, -m))
    assert len(lst) == NCH
    return lst


def kernel(cos_theta: np.ndarray, phi: np.ndarray):
    nc = _get_nc()
    x = np.asarray(cos_theta, dtype=np.float32).reshape(N_CORES, P, W)
    p = np.asarray(phi, dtype=np.float32).reshape(N_CORES, P, W)
    salt = np.zeros((1, nc._angular_salt_w), np.float32)
    in_maps = [{"x": x[i], "phi": p[i], "salt": salt} for i in range(N_CORES)]
    res = run_bass_kernel_spmd(nc, in_maps, core_ids=list(range(N_CORES)))
    outs = res.results
    # outs: per core "out" is [121, P, W] in production order
    order = _production_order()
    chan_of_seq = np.array([_chan(l, m) for (l, m) in order])
    perm = np.empty(NCH, dtype=np.int64)
    perm[chan_of_seq] = np.arange(NCH)  # perm[chan] = seq index
    full = np.empty((N_PTS, NCH), dtype=np.float32)
    for i in range(N_CORES):
        o = np.asarray(outs[i]["out"]).astype(np.float32)  # [11, P, 11, W]
        o = o.transpose(0, 2, 1, 3).reshape(NCH, P * W)     # seq-major
        full[i * P * W:(i + 1) * P * W] = o.T[:, perm]
    result = []
    for l in range(L_MAX + 1):
        result.append(np.ascontiguousarray(full[:, l * l:(l + 1) * (l + 1)]))
    return tuple(result)
